# revision 31
# baseline (speedup 1.0000x reference)
"""RWKV-v4 block (time-mix WKV + channel-mix GLU) on 8 TRN2 NeuronCores,
data-parallel over batch B.

All-layout-B design: activations live as [c(128p) x 4 chunks, t(1024)] bf16
tiles; the host pre-transposes x to [B, C, T] bf16 and transposes the bf16
output back (pure data movement, no host FLOPs).

- The time-shift mixes (xk = tm*xn_t + (1-tm)*xn_{t-1}, etc.) are folded into
  the matmuls: host pre-splits every xn-consuming weight into A = W*diag(tm)
  and B = W*diag(1-tm); the matmul accumulates A @ xn_t + B @ xn_{t-1} where
  xn_{t-1} is just a one-column-shifted AP view of the same fp8 xn tile
  (zero column at t=0).  This removes all mix/delta elementwise work.
- LayerNorm stats via an all-ones [128,128] bf16 stationary matmul: sum and
  sum-of-squares land replicated across every partition (broadcast for free).
  rstd = exp(-0.5*ln(var+eps)) keeps every ACT func (exp/ln/square/copy/relu)
  in the natural_log_exp table -> zero table reloads.  Sigmoids are
  1/(1+exp(-x)) with the reciprocal fused into the WKV normalizer / GLU gate.
- All matmuls are fp8e4 DoubleRow (2 contraction rows packed per partition):
  weights are host-scaled x64 (cWk x4 to keep relu(k)^2 in fp8 range) and the
  scales folded back out through psum-read `scale=` args downstream.
- WKV recurrence (per 128-channel chunk, scan along t):
    P'_t = d*P'_{t-1} + (e*v64)_t   Q_t = d*Q_{t-1} + e_t     e = exp(k)
    s64  = (eu*e*v64 + P'_{t-1}) / ((Q_{t-1} + eu*e) * (1 + exp(-r)))
  (64x scale rides through linearly; folded out at the Wo residual add).
  The scan decay multiplier stays f32: bf16 error in d~0.993 would compound
  exponentially over T.  Scan carries are f32 internally regardless.
- ln1_b/ln2_b are asserted zero (holds for this model init), which zeroes the
  Wv/Wr/cWk bias projections and lets exp read psum directly.
- Engine split: PE matmuls+stats; DVE scans/recips/stt chains; ACT exp/ln/
  square/psum-drain copies; Pool(gpsimd) sbuf-only tt (evu', Dt, t1, squares).
"""

import numpy as np
import ml_dtypes
from contextlib import ExitStack

import concourse.bass as bass
import concourse.tile as tile
from concourse import bacc, mybir

B, T, C = 32, 1024, 512
H = 4 * C
NCORES = 8
BL = B // NCORES  # batches per core
CC = C // 128     # 4 channel chunks
HC = H // 128     # 16 hidden chunks

F32 = mybir.dt.float32
BF16 = mybir.dt.bfloat16
FP8 = mybir.dt.float8e4
OP = mybir.AluOpType
AF = mybir.ActivationFunctionType
PM = mybir.MatmulPerfMode

WS = 64.0   # fp8 weight scale (all but cWk)
KS = 4.0    # cWk fp8 scale; kk8 = (KS*khat)^2 = 16*kk stays < 240


def _emit(nc, tc, ctx, io, bl):
    x_d = io["x"].ap()
    y_d = io["y"].ap()

    def col(name, c0):  # [128,1] slice of a [N] dram vector
        return io[name].ap()[c0 * 128:(c0 + 1) * 128].rearrange(
            "(c one) -> c one", one=1)

    wp = ctx.enter_context(tc.tile_pool(name="wp", bufs=1))

    def load_pairs(name, npairs, cols):
        ts_ = []
        for j in range(npairs):
            t_ = wp.tile([128, 2, cols], FP8, tag=f"w_{name}_{j}")
            nc.sync.dma_start(t_[:], io[name].ap()[j])
            ts_.append(t_)
        return ts_

    wk8a = load_pairs("wk8a", 2, C)
    wk8b = load_pairs("wk8b", 2, C)
    wv8a = load_pairs("wv8a", 2, C)
    wv8b = load_pairs("wv8b", 2, C)
    wr8a = load_pairs("wr8a", 2, C)
    wr8b = load_pairs("wr8b", 2, C)
    wo8 = load_pairs("wo8", 2, C)
    cwk8a = load_pairs("cwk8a", 2, H)
    cwk8b = load_pairs("cwk8b", 2, H)
    cwv8 = load_pairs("cwv8", 8, C)
    cwr8a = load_pairs("cwr8a", 2, C)
    cwr8b = load_pairs("cwr8b", 2, C)

    def vec4(name):
        ts_ = []
        for i in range(CC):
            t_ = wp.tile([128, 1], F32, tag=f"v_{name}_{i}")
            nc.sync.dma_start(t_[:], col(name, i))
            ts_.append(t_)
        return ts_

    eu_c = vec4("eu")

    # materialized [128,512] f32 decay tiles (stride-0 broadcast APs make
    # scans ~40% slower; f32 keeps the decay exact)
    delta_c = vec4("delta")
    dbt = []
    for i in range(CC):
        t_ = wp.tile([128, 512], F32, tag=f"dbt_{i}")
        nc.scalar.activation(t_[:], delta_c[i][:].to_broadcast((128, 512)),
                             AF.Copy)
        dbt.append(t_)

    ones128 = wp.tile([128, 128], BF16, tag="ones128")
    nc.vector.memset(ones128[:], 1.0)
    eps_t = wp.tile([128, 1], F32, tag="eps")
    nc.vector.memset(eps_t[:], 1e-5)
    c16_t = wp.tile([128, 1], F32, tag="c16")
    nc.vector.memset(c16_t[:], WS * KS * KS)
    one_t = wp.tile([128, 1], F32, tag="one")
    nc.vector.memset(one_t[:], 1.0)

    # ---- per-batch pools ----
    xp = ctx.enter_context(tc.tile_pool(name="xp", bufs=2))       # x tiles
    x1p = ctx.enter_context(tc.tile_pool(name="x1p", bufs=2))     # x1 tiles
    sqp = ctx.enter_context(tc.tile_pool(name="sqp", bufs=2))     # scratch
    lnp = ctx.enter_context(tc.tile_pool(name="lnp", bufs=1))     # xn8/rstd/nmr
    wkp = ctx.enter_context(tc.tile_pool(name="wkp", bufs=2))     # wkv transients
    wkq = ctx.enter_context(tc.tile_pool(name="wkq", bufs=2))     # wkv tail
    sp_ = ctx.enter_context(tc.tile_pool(name="sp", bufs=1))      # s' fp8
    kkp = ctx.enter_context(tc.tile_pool(name="kkp", bufs=1))     # kk fp8
    cmp_ = ctx.enter_context(tc.tile_pool(name="cmp", bufs=2))    # cm transients
    ps = ctx.enter_context(tc.tile_pool(name="ps", bufs=3, space="PSUM"))
    pst = ctx.enter_context(tc.tile_pool(name="pst", bufs=1, space="PSUM"))

    def layer_norm(xt, pf):
        """xt: 4x [128, T] bf16 chunk tiles -> xn8: 2x [128, 2, T+1] fp8
        pair tiles (zero col at t=0; slot i of pair j = channel chunk 2j+i).
        Stats via the all-ones stationary (replicated across partitions);
        rstd = rsqrt(var) by exponent bit-seed + one Newton step (no Ln/Sqrt
        funcs -> single ACT table).  eps is dropped: var >= O(0.1) here."""
        var = lnp.tile([128, T], F32, tag=pf + "var")
        mb = lnp.tile([128, T], BF16, tag=pf + "mb")
        for tqh in range(2):
            sts = [pst.tile([128, 2, 256], F32, tag=f"st{q}", name=f"st{q}")
                   for q in range(2)]
            for cc in range(CC):
                for q in range(2):
                    tq = 2 * tqh + q
                    nc.tensor.matmul(sts[q][:, 0, :], ones128[:],
                                     xt[cc][:, tq * 256:(tq + 1) * 256],
                                     start=(cc == 0), stop=(cc == CC - 1))
            for cc in range(CC):
                scr = sqp.tile([128, 512], BF16, tag="sq")
                nc.scalar.activation(scr[:],
                                     xt[cc][:, tqh * 512:(tqh + 1) * 512],
                                     AF.Square)
                for q in range(2):
                    nc.tensor.matmul(sts[q][:, 1, :], ones128[:],
                                     scr[:, q * 256:(q + 1) * 256],
                                     start=(cc == 0), stop=(cc == CC - 1))
            for q in range(2):
                st = sts[q]
                tq = 2 * tqh + q
                sl = slice(tq * 256, (tq + 1) * 256)
                msq = lnp.tile([128, 256], BF16, tag=pf + f"msq{q}")
                nc.scalar.activation(msq[:], st[:, 0, :], AF.Square,
                                     scale=1.0 / C)
                nc.vector.scalar_tensor_tensor(var[:, sl], st[:, 1, :],
                                               1.0 / C, msq[:], op0=OP.mult,
                                               op1=OP.subtract)
                nc.scalar.activation(mb[:, sl], st[:, 0, :], AF.Copy,
                                     scale=1.0 / C)
        # rstd = rsqrt(var): seed = bitcast(0x5f3759df - (bits >> 1)), then
        # y1 = y0*(1.5 - 0.5*var*y0^2)
        sh = lnp.tile([128, T], mybir.dt.int32, tag=pf + "sh")
        nc.vector.tensor_scalar(sh[:], var[:].bitcast(mybir.dt.int32), 1,
                                None, op0=OP.arith_shift_right)
        nc.vector.tensor_scalar(sh[:], sh[:], -1, 0x5f3759df, op0=OP.mult,
                                op1=OP.add)
        y0 = sh[:].bitcast(mybir.dt.float32)
        ysq = lnp.tile([128, T], BF16, tag=pf + "ysq")
        nc.scalar.activation(ysq[:], y0, AF.Square)
        nc.gpsimd.tensor_tensor(ysq[:], ysq[:], var[:], op=OP.mult)
        nc.gpsimd.tensor_scalar(ysq[:], ysq[:], -0.5, 1.5, op0=OP.mult,
                                op1=OP.add)
        rstd = lnp.tile([128, T], BF16, tag=pf + "rstd")
        nc.gpsimd.tensor_tensor(rstd[:], ysq[:], y0, op=OP.mult)
        # width T+2: even slot stride (odd strides break PE moving fetch)
        xn8 = [lnp.tile([128, 2, T + 2], FP8, tag=pf + f"xn8_{j}", name=pf + f"xn8_{j}")
               for j in range(2)]
        for j in range(2):
            nc.vector.memset(xn8[j][:, :, 0:1], 0.0)
        for cc in range(CC):
            tmp = lnp.tile([128, T], BF16, tag=pf + "lntmp")
            eng = nc.gpsimd if cc % 2 == 0 else nc.vector
            eng.tensor_tensor(tmp[:], xt[cc][:], mb[:], op=OP.subtract)
            nc.vector.tensor_tensor(xn8[cc // 2][:, cc % 2, 1:T + 1], tmp[:],
                                    rstd[:], op=OP.mult)
        return xn8

    def mm_shift(wa, wb, xn8, mcol):
        """out[:, th*512:] = sum_j (A_j @ xn_t + B_j @ xn_{t-1}); xn_{t-1} is
        the one-column-left view of the same fp8 tile."""
        out = ps.tile([128, T], F32, tag="ps")
        wlist = [(wa, 1), (wb, 0)]
        for j in range(2):
            for wi, (w, off) in enumerate(wlist):
                for th in range(2):
                    nc.tensor.matmul(
                        out[:, th * 512:(th + 1) * 512],
                        w[j][:, :, mcol * 128:(mcol + 1) * 128],
                        xn8[j][:, :, off + th * 512:off + th * 512 + 512],
                        start=(j == 0 and wi == 0),
                        stop=(j == 1 and wi == 1),
                        perf_mode=PM.DoubleRow)
        return out

    def mm_pair(wtiles, xtiles, mcol):
        out = ps.tile([128, T], F32, tag="ps")
        nj = len(wtiles)
        for j in range(nj):
            for th in range(2):
                nc.tensor.matmul(out[:, th * 512:(th + 1) * 512],
                                 wtiles[j][:, :, mcol * 128:(mcol + 1) * 128],
                                 xtiles[j][:, :, th * 512:(th + 1) * 512],
                                 start=(j == 0), stop=(j == nj - 1),
                                 perf_mode=PM.DoubleRow)
        return out

    # ---- per-batch phases, emitted with a 2-step skew so the scheduler can
    # overlap batch b's WKV/CM with batch b+1's LN/kvr ----
    st = [dict() for _ in range(bl)]

    def p0(b):   # load + LN1
        xt = []
        for cc in range(CC):
            t_ = xp.tile([128, T], BF16, tag=f"x{cc}")
            nc.sync.dma_start(t_[:], x_d[b, cc * 128:(cc + 1) * 128, :])
            xt.append(t_)
        st[b]["xt"] = xt
        st[b]["xn8"] = layer_norm(xt, "a")

    def wkv_half(b, hhs):
        xn8 = st[b]["xn8"]
        s8 = st[b]["s8"]
        for hh in hhs:
            kp = mm_shift(wk8a, wk8b, xn8, hh)
            e = wkp.tile([128, T], BF16, tag="e")
            nc.scalar.activation(e[:], kp[:], AF.Exp, scale=1.0 / WS)

            Qb = wkp.tile([128, T + 1], BF16, tag="Qb")
            Pb = wkp.tile([128, T + 1], BF16, tag="Pb")
            nc.gpsimd.memset(Qb[:, 0:1], 0.0)
            nc.vector.memset(Pb[:, 0:1], 0.0)
            nc.vector.tensor_tensor_scan(Qb[:, 1:513], dbt[hh][:],
                                         e[:, 0:512], 0.0, op0=OP.mult,
                                         op1=OP.add)
            nc.vector.tensor_tensor_scan(Qb[:, 513:T + 1], dbt[hh][:],
                                         e[:, 512:T], Qb[:, 512:513],
                                         op0=OP.mult, op1=OP.add)
            Dt = wkp.tile([128, T], BF16, tag="eue")
            nc.vector.scalar_tensor_tensor(Dt[:], e[:], eu_c[hh][:],
                                           Qb[:, 0:T], op0=OP.mult,
                                           op1=OP.add)
            rp = mm_shift(wr8a, wr8b, xn8, hh)
            es1 = wkq.tile([128, T], BF16, tag="es1")
            nc.scalar.activation(es1[:], rp[:], AF.Exp, scale=-1.0 / WS)
            vp = mm_shift(wv8a, wv8b, xn8, hh)
            nc.vector.tensor_tensor(e[:], e[:], vp[:], op=OP.mult)  # ev'
            nc.vector.tensor_tensor_scan(Pb[:, 1:513], dbt[hh][:],
                                         e[:, 0:512], 0.0, op0=OP.mult,
                                         op1=OP.add)
            nc.vector.tensor_tensor_scan(Pb[:, 513:T + 1], dbt[hh][:],
                                         e[:, 512:T], Pb[:, 512:513],
                                         op0=OP.mult, op1=OP.add)
            Np = wkp.tile([128, T], BF16, tag="evu")
            nc.vector.scalar_tensor_tensor(Np[:], e[:], eu_c[hh][:],
                                           Pb[:, 0:T], op0=OP.mult,
                                           op1=OP.add)
            D2 = wkq.tile([128, T], F32, tag="D2")
            nc.vector.scalar_tensor_tensor(D2[:], es1[:], 1.0, Dt[:],
                                           op0=OP.add, op1=OP.mult)
            nc.vector.reciprocal_approx_fast(D2[:], D2[:])
            nc.gpsimd.tensor_tensor(s8[hh // 2][:, hh % 2, :], Np[:], D2[:],
                                    op=OP.mult)

    def p1a(b):
        st[b]["s8"] = [sp_.tile([128, 2, T], FP8, tag=f"s8_{j}",
                                name=f"s8_{j}") for j in range(2)]
        wkv_half(b, (0, 1))

    def p1b(b):
        wkv_half(b, (2, 3))

    def p2(b):   # Wo + residual + LN2
        xt, s8 = st[b]["xt"], st[b]["s8"]
        x1t = []
        for cc in range(CC):
            wop = mm_pair(wo8, s8, cc)
            t_ = x1p.tile([128, T], BF16, tag=f"x1_{cc}")
            nc.vector.scalar_tensor_tensor(t_[:], wop[:], 1.0 / (WS * WS),
                                           xt[cc][:], op0=OP.mult, op1=OP.add)
            x1t.append(t_)
        st[b]["x1t"] = x1t
        st[b]["xn28"] = layer_norm(x1t, "b")

    def kk_half(b, hhs):
        xn28, kk8 = st[b]["xn28"], st[b]["kk8"]
        for hh in hhs:
            ckp = mm_shift(cwk8a, cwk8b, xn28, hh)
            khr = sqp.tile([128, T], BF16, tag="khr")
            nc.scalar.activation(khr[:], ckp[:], AF.Relu)
            if hh % 2 == 0:
                nc.gpsimd.tensor_tensor(kk8[hh // 2][:, hh % 2, :], khr[:],
                                        khr[:], op=OP.mult)
            else:
                nc.scalar.activation(kk8[hh // 2][:, hh % 2, :], khr[:],
                                     AF.Square)

    def p3a(b):
        st[b]["kk8"] = [kkp.tile([128, 2, T], FP8, tag=f"kk8_{j}",
                                 name=f"kk8_{j}") for j in range(HC // 2)]
        kk_half(b, range(0, 8))

    def p3b(b):
        kk_half(b, range(8, HC))

    def p4(b):   # gate + cwv + output
        xn28, kk8, x1t = st[b]["xn28"], st[b]["kk8"], st[b]["x1t"]
        for cc in range(CC):
            r2p = mm_shift(cwr8a, cwr8b, xn28, cc)
            es2 = cmp_.tile([128, T], BF16, tag="es2")
            nc.scalar.activation(es2[:], r2p[:], AF.Exp, scale=-1.0 / WS)
            sig2 = cmp_.tile([128, T], F32, tag="sig2")
            nc.scalar.activation(sig2[:], es2[:], AF.Identity, bias=c16_t[:],
                                 scale=WS * KS * KS)
            nc.vector.reciprocal_approx_fast(sig2[:], sig2[:])
            kvp = mm_pair(cwv8, kk8, cc)
            t1 = cmp_.tile([128, T], BF16, tag="t1")
            nc.vector.scalar_tensor_tensor(t1[:], kvp[:], 1.0, sig2[:],
                                           op0=OP.mult, op1=OP.mult)
            nc.gpsimd.tensor_tensor(t1[:], t1[:], x1t[cc][:], op=OP.add)
            nc.sync.dma_start(y_d[b, cc * 128:(cc + 1) * 128, :], t1[:])

    phases = [p0, p1a, p1b, p2, p3a, p3b, p4]
    NP = len(phases)
    SKEW = 2
    for t in range(NP + SKEW * (bl - 1)):
        for b in range(bl - 1, -1, -1):   # older batches first? no: higher b
            pass
        for b in range(bl):
            p = t - SKEW * b
            if 0 <= p < 5:
                pass
        # emit descending phase index (older batch first) within the step
        work = [(t - SKEW * b, b) for b in range(bl)
                if 0 <= t - SKEW * b < NP]
        for p, b in sorted(work, reverse=True):
            phases[p](b)


def build_program(bl=BL):
    nc = bacc.Bacc("TRN2", target_bir_lowering=False, debug=False,
                   num_devices=NCORES)
    io = {}
    io["x"] = nc.dram_tensor("x", [bl, C, T], BF16, kind="ExternalInput")
    io["y"] = nc.dram_tensor("y", [bl, C, T], BF16, kind="ExternalOutput")
    for nm, npairs, cols in [("wk8a", 2, C), ("wk8b", 2, C), ("wv8a", 2, C),
                             ("wv8b", 2, C), ("wr8a", 2, C), ("wr8b", 2, C),
                             ("wo8", 2, C), ("cwk8a", 2, H), ("cwk8b", 2, H),
                             ("cwv8", 8, C), ("cwr8a", 2, C),
                             ("cwr8b", 2, C)]:
        io[nm] = nc.dram_tensor(nm, [npairs, 128, 2, cols], FP8,
                                kind="ExternalInput")
    for nm in ["delta", "eu"]:
        io[nm] = nc.dram_tensor(nm, [C], F32, kind="ExternalInput")

    with tile.TileContext(nc) as tc:
        with ExitStack() as ctx:
            _emit(nc, tc, ctx, io, bl)
    nc.compile()
    return nc


def _pack_pairs(wT, scale):
    """wT [K, M] contraction-major -> [K//256, 128, 2, M] fp8, slot i of pair
    j = contraction chunk 2j+i."""
    K, M = wT.shape
    out = np.empty((K // 256, 128, 2, M), np.float32)
    for j in range(K // 256):
        for i in range(2):
            out[j, :, i, :] = wT[(2 * j + i) * 128:(2 * j + i + 1) * 128, :]
    out = np.clip(out * scale, -224.0, 224.0)
    return np.ascontiguousarray(out.astype(ml_dtypes.float8_e4m3))


def host_params(inputs):
    """Host-side parameter prep (O(C^2) transposes/folds only)."""
    f32 = np.float32
    g1 = np.asarray(inputs["ln1_g"], f32)
    b1 = np.asarray(inputs["ln1_b"], f32)
    g2 = np.asarray(inputs["ln2_g"], f32)
    b2 = np.asarray(inputs["ln2_b"], f32)
    assert np.allclose(b1, 0.0, atol=1e-30), "nonzero ln1_b not supported"
    assert np.allclose(b2, 0.0, atol=1e-30), "nonzero ln2_b not supported"
    Wk = np.asarray(inputs["Wk"], f32)
    Wv = np.asarray(inputs["Wv"], f32)
    Wr = np.asarray(inputs["Wr"], f32)
    Wo = np.asarray(inputs["Wo"], f32)
    cWk = np.asarray(inputs["cWk"], f32)
    cWr = np.asarray(inputs["cWr"], f32)
    cWv = np.asarray(inputs["cWv"], f32)
    tmk = np.asarray(inputs["tm_k"], f32)[:, None]
    tmv = np.asarray(inputs["tm_v"], f32)[:, None]
    tmr = np.asarray(inputs["tm_r"], f32)[:, None]
    cmk = np.asarray(inputs["cm_k"], f32)[:, None]
    cmr = np.asarray(inputs["cm_r"], f32)[:, None]

    wkT = Wk.T * g1[:, None]
    wvT = Wv.T * g1[:, None]
    wrT = Wr.T * g1[:, None]
    cwkT = cWk.T * g2[:, None]
    cwrT = cWr.T * g2[:, None]

    p = {
        "wk8a": _pack_pairs(wkT * tmk, WS),
        "wk8b": _pack_pairs(wkT * (1.0 - tmk), WS),
        "wv8a": _pack_pairs(wvT * tmv, WS),
        "wv8b": _pack_pairs(wvT * (1.0 - tmv), WS),
        "wr8a": _pack_pairs(wrT * tmr, WS),
        "wr8b": _pack_pairs(wrT * (1.0 - tmr), WS),
        "wo8": _pack_pairs(Wo.T, WS),
        "cwk8a": _pack_pairs(cwkT * cmk, KS),
        "cwk8b": _pack_pairs(cwkT * (1.0 - cmk), KS),
        "cwv8": _pack_pairs(cWv.T, WS),
        "cwr8a": _pack_pairs(cwrT * cmr, WS),
        "cwr8b": _pack_pairs(cwrT * (1.0 - cmr), WS),
        "delta": np.exp(-np.exp(np.asarray(inputs["time_decay"], f32))),
        "eu": np.exp(np.asarray(inputs["time_first"], f32)),
    }
    return p


def host_x(x_sub):
    """[n, T, C] f32 -> [n, C, T] bf16 (layout B)."""
    return np.ascontiguousarray(
        x_sub.transpose(0, 2, 1).astype(ml_dtypes.bfloat16))


def host_y(y_dev):
    """[n, C, T] bf16 -> [n, T, C] f32."""
    return np.asarray(y_dev).astype(np.float32).transpose(0, 2, 1)


def make_in_maps(inputs):
    p = host_params(inputs)
    x = np.asarray(inputs["x"], np.float32)
    return [dict(p, x=host_x(x[c * BL:(c + 1) * BL])) for c in range(NCORES)]


_CACHE = {}


def kernel(**inputs):
    from concourse.bass_utils import run_bass_kernel_spmd

    if "nc" not in _CACHE:
        _CACHE["nc"] = build_program(BL)
    nc = _CACHE["nc"]

    in_maps = make_in_maps(inputs)
    res = run_bass_kernel_spmd(nc, in_maps, list(range(NCORES)))
    out = np.concatenate([host_y(res.results[c]["y"]) for c in range(NCORES)],
                         axis=0)
    return np.ascontiguousarray(out.astype(np.float32))


# revision 32
# speedup vs baseline: 1.0284x; 1.0284x over previous
"""RWKV-v4 block (time-mix WKV + channel-mix GLU) on 8 TRN2 NeuronCores,
data-parallel over batch B.

All-layout-B design: activations live as [c(128p) x 4 chunks, t(1024)] bf16
tiles; the host pre-transposes x to [B, C, T] bf16 and transposes the bf16
output back (pure data movement, no host FLOPs).

- The time-shift mixes (xk = tm*xn_t + (1-tm)*xn_{t-1}, etc.) are folded into
  the matmuls: host pre-splits every xn-consuming weight into A = W*diag(tm)
  and B = W*diag(1-tm); the matmul accumulates A @ xn_t + B @ xn_{t-1} where
  xn_{t-1} is just a one-column-shifted AP view of the same fp8 xn tile
  (zero column at t=0).  This removes all mix/delta elementwise work.
- LayerNorm stats via an all-ones [128,128] bf16 stationary matmul: sum and
  sum-of-squares land replicated across every partition (broadcast for free).
  rstd = exp(-0.5*ln(var+eps)) keeps every ACT func (exp/ln/square/copy/relu)
  in the natural_log_exp table -> zero table reloads.  Sigmoids are
  1/(1+exp(-x)) with the reciprocal fused into the WKV normalizer / GLU gate.
- All matmuls are fp8e4 DoubleRow (2 contraction rows packed per partition):
  weights are host-scaled x64 (cWk x4 to keep relu(k)^2 in fp8 range) and the
  scales folded back out through psum-read `scale=` args downstream.
- WKV recurrence (per 128-channel chunk, scan along t):
    P'_t = d*P'_{t-1} + (e*v64)_t   Q_t = d*Q_{t-1} + e_t     e = exp(k)
    s64  = (eu*e*v64 + P'_{t-1}) / ((Q_{t-1} + eu*e) * (1 + exp(-r)))
  (64x scale rides through linearly; folded out at the Wo residual add).
  The scan decay multiplier stays f32: bf16 error in d~0.993 would compound
  exponentially over T.  Scan carries are f32 internally regardless.
- ln1_b/ln2_b are asserted zero (holds for this model init), which zeroes the
  Wv/Wr/cWk bias projections and lets exp read psum directly.
- Engine split: PE matmuls+stats; DVE scans/recips/stt chains; ACT exp/ln/
  square/psum-drain copies; Pool(gpsimd) sbuf-only tt (evu', Dt, t1, squares).
"""

import numpy as np
import ml_dtypes
from contextlib import ExitStack

import concourse.bass as bass
import concourse.tile as tile
from concourse import bacc, mybir

B, T, C = 32, 1024, 512
H = 4 * C
NCORES = 8
BL = B // NCORES  # batches per core
CC = C // 128     # 4 channel chunks
HC = H // 128     # 16 hidden chunks

F32 = mybir.dt.float32
BF16 = mybir.dt.bfloat16
FP8 = mybir.dt.float8e4
OP = mybir.AluOpType
AF = mybir.ActivationFunctionType
PM = mybir.MatmulPerfMode

WS = 64.0   # fp8 weight scale (all but cWk)
KS = 4.0    # cWk fp8 scale; kk8 = (KS*khat)^2 = 16*kk stays < 240


def _emit(nc, tc, ctx, io, bl):
    x_d = io["x"].ap()
    y_d = io["y"].ap()

    def col(name, c0):  # [128,1] slice of a [N] dram vector
        return io[name].ap()[c0 * 128:(c0 + 1) * 128].rearrange(
            "(c one) -> c one", one=1)

    wp = ctx.enter_context(tc.tile_pool(name="wp", bufs=1))

    def load_pairs(name, npairs, cols):
        ts_ = []
        for j in range(npairs):
            t_ = wp.tile([128, 2, cols], FP8, tag=f"w_{name}_{j}")
            nc.sync.dma_start(t_[:], io[name].ap()[j])
            ts_.append(t_)
        return ts_

    wk8a = load_pairs("wk8a", 2, C)
    wk8b = load_pairs("wk8b", 2, C)
    wv8a = load_pairs("wv8a", 2, C)
    wv8b = load_pairs("wv8b", 2, C)
    wr8a = load_pairs("wr8a", 2, C)
    wr8b = load_pairs("wr8b", 2, C)
    wo8 = load_pairs("wo8", 2, C)
    cwk8a = load_pairs("cwk8a", 2, H)
    cwk8b = load_pairs("cwk8b", 2, H)
    cwv8 = load_pairs("cwv8", 8, C)
    cwr8a = load_pairs("cwr8a", 2, C)
    cwr8b = load_pairs("cwr8b", 2, C)

    def vec4(name):
        ts_ = []
        for i in range(CC):
            t_ = wp.tile([128, 1], F32, tag=f"v_{name}_{i}")
            nc.sync.dma_start(t_[:], col(name, i))
            ts_.append(t_)
        return ts_

    eu_c = vec4("eu")

    # materialized [128,512] f32 decay tiles (stride-0 broadcast APs make
    # scans ~40% slower; f32 keeps the decay exact)
    delta_c = vec4("delta")
    dbt = []
    for i in range(CC):
        t_ = wp.tile([128, 512], F32, tag=f"dbt_{i}")
        nc.scalar.activation(t_[:], delta_c[i][:].to_broadcast((128, 512)),
                             AF.Copy)
        dbt.append(t_)

    ones128 = wp.tile([128, 128], BF16, tag="ones128")
    nc.vector.memset(ones128[:], 1.0)
    eps_t = wp.tile([128, 1], F32, tag="eps")
    nc.vector.memset(eps_t[:], 1e-5)
    c16_t = wp.tile([128, 1], F32, tag="c16")
    nc.vector.memset(c16_t[:], WS * KS * KS)
    one_t = wp.tile([128, 1], F32, tag="one")
    nc.vector.memset(one_t[:], 1.0)

    # ---- per-batch pools ----
    xp = ctx.enter_context(tc.tile_pool(name="xp", bufs=2))       # x tiles
    x1p = ctx.enter_context(tc.tile_pool(name="x1p", bufs=2))     # x1 tiles
    sqp = ctx.enter_context(tc.tile_pool(name="sqp", bufs=2))     # scratch
    lnp = ctx.enter_context(tc.tile_pool(name="lnp", bufs=1))     # xn8/rstd/nmr
    wkp = ctx.enter_context(tc.tile_pool(name="wkp", bufs=2))     # wkv transients
    wkq = ctx.enter_context(tc.tile_pool(name="wkq", bufs=2))     # wkv tail
    sp_ = ctx.enter_context(tc.tile_pool(name="sp", bufs=1))      # s' fp8
    kkp = ctx.enter_context(tc.tile_pool(name="kkp", bufs=1))     # kk fp8
    cmp_ = ctx.enter_context(tc.tile_pool(name="cmp", bufs=2))    # cm transients
    ps = ctx.enter_context(tc.tile_pool(name="ps", bufs=3, space="PSUM"))
    pst = ctx.enter_context(tc.tile_pool(name="pst", bufs=1, space="PSUM"))

    def layer_norm(xt, pf):
        """xt: 4x [128, T] bf16 chunk tiles -> xn8: 2x [128, 2, T+1] fp8
        pair tiles (zero col at t=0; slot i of pair j = channel chunk 2j+i).
        Stats via the all-ones stationary (replicated across partitions);
        rstd = rsqrt(var) by exponent bit-seed + one Newton step (no Ln/Sqrt
        funcs -> single ACT table).  eps is dropped: var >= O(0.1) here."""
        var = lnp.tile([128, T], F32, tag=pf + "var")
        mb = lnp.tile([128, T], BF16, tag=pf + "mb")
        for tqh in range(2):
            sts = [pst.tile([128, 2, 256], F32, tag=f"st{q}", name=f"st{q}")
                   for q in range(2)]
            for cc in range(CC):
                for q in range(2):
                    tq = 2 * tqh + q
                    nc.tensor.matmul(sts[q][:, 0, :], ones128[:],
                                     xt[cc][:, tq * 256:(tq + 1) * 256],
                                     start=(cc == 0), stop=(cc == CC - 1))
            for cc in range(CC):
                scr = sqp.tile([128, 512], BF16, tag="sq")
                nc.vector.tensor_tensor(scr[:],
                                        xt[cc][:, tqh * 512:(tqh + 1) * 512],
                                        xt[cc][:, tqh * 512:(tqh + 1) * 512],
                                        op=OP.mult)
                for q in range(2):
                    nc.tensor.matmul(sts[q][:, 1, :], ones128[:],
                                     scr[:, q * 256:(q + 1) * 256],
                                     start=(cc == 0), stop=(cc == CC - 1))
            for q in range(2):
                st = sts[q]
                tq = 2 * tqh + q
                sl = slice(tq * 256, (tq + 1) * 256)
                msq = lnp.tile([128, 256], BF16, tag=pf + f"msq{q}")
                nc.scalar.activation(msq[:], st[:, 0, :], AF.Square,
                                     scale=1.0 / C)
                nc.vector.scalar_tensor_tensor(var[:, sl], st[:, 1, :],
                                               1.0 / C, msq[:], op0=OP.mult,
                                               op1=OP.subtract)
                nc.scalar.activation(mb[:, sl], st[:, 0, :], AF.Copy,
                                     scale=1.0 / C)
        # rstd = rsqrt(var): seed = bitcast(0x5f3759df - (bits >> 1)), then
        # y1 = y0*(1.5 - 0.5*var*y0^2)
        sh = lnp.tile([128, T], mybir.dt.int32, tag=pf + "sh")
        nc.vector.tensor_scalar(sh[:], var[:].bitcast(mybir.dt.int32), 1,
                                None, op0=OP.arith_shift_right)
        nc.vector.tensor_scalar(sh[:], sh[:], -1, 0x5f3759df, op0=OP.mult,
                                op1=OP.add)
        y0 = sh[:].bitcast(mybir.dt.float32)
        ysq = lnp.tile([128, T], BF16, tag=pf + "ysq")
        nc.scalar.activation(ysq[:], y0, AF.Square)
        nc.gpsimd.tensor_tensor(ysq[:], ysq[:], var[:], op=OP.mult)
        nc.gpsimd.tensor_scalar(ysq[:], ysq[:], -0.5, 1.5, op0=OP.mult,
                                op1=OP.add)
        rstd = lnp.tile([128, T], BF16, tag=pf + "rstd")
        nc.gpsimd.tensor_tensor(rstd[:], ysq[:], y0, op=OP.mult)
        # width T+2: even slot stride (odd strides break PE moving fetch)
        xn8 = [lnp.tile([128, 2, T + 2], FP8, tag=pf + f"xn8_{j}", name=pf + f"xn8_{j}")
               for j in range(2)]
        for j in range(2):
            nc.vector.memset(xn8[j][:, :, 0:1], 0.0)
        for cc in range(CC):
            tmp = lnp.tile([128, T], BF16, tag=pf + "lntmp")
            eng = nc.gpsimd if cc % 2 == 0 else nc.vector
            eng.tensor_tensor(tmp[:], xt[cc][:], mb[:], op=OP.subtract)
            nc.vector.tensor_tensor(xn8[cc // 2][:, cc % 2, 1:T + 1], tmp[:],
                                    rstd[:], op=OP.mult)
        return xn8

    def mm_shift(wa, wb, xn8, mcol):
        """out[:, th*512:] = sum_j (A_j @ xn_t + B_j @ xn_{t-1}); xn_{t-1} is
        the one-column-left view of the same fp8 tile."""
        out = ps.tile([128, T], F32, tag="ps")
        wlist = [(wa, 1), (wb, 0)]
        for j in range(2):
            for wi, (w, off) in enumerate(wlist):
                for th in range(2):
                    nc.tensor.matmul(
                        out[:, th * 512:(th + 1) * 512],
                        w[j][:, :, mcol * 128:(mcol + 1) * 128],
                        xn8[j][:, :, off + th * 512:off + th * 512 + 512],
                        start=(j == 0 and wi == 0),
                        stop=(j == 1 and wi == 1),
                        perf_mode=PM.DoubleRow)
        return out

    def mm_pair(wtiles, xtiles, mcol):
        out = ps.tile([128, T], F32, tag="ps")
        nj = len(wtiles)
        for j in range(nj):
            for th in range(2):
                nc.tensor.matmul(out[:, th * 512:(th + 1) * 512],
                                 wtiles[j][:, :, mcol * 128:(mcol + 1) * 128],
                                 xtiles[j][:, :, th * 512:(th + 1) * 512],
                                 start=(j == 0), stop=(j == nj - 1),
                                 perf_mode=PM.DoubleRow)
        return out

    # ---- per-batch phases, emitted with a 2-step skew so the scheduler can
    # overlap batch b's WKV/CM with batch b+1's LN/kvr ----
    st = [dict() for _ in range(bl)]

    def p0(b):   # load + LN1
        xt = []
        for cc in range(CC):
            t_ = xp.tile([128, T], BF16, tag=f"x{cc}")
            nc.sync.dma_start(t_[:], x_d[b, cc * 128:(cc + 1) * 128, :])
            xt.append(t_)
        st[b]["xt"] = xt
        st[b]["xn8"] = layer_norm(xt, "a")

    def wkv_half(b, hhs):
        xn8 = st[b]["xn8"]
        s8 = st[b]["s8"]
        for hh in hhs:
            kp = mm_shift(wk8a, wk8b, xn8, hh)
            e = wkp.tile([128, T], BF16, tag="e")
            nc.scalar.activation(e[:], kp[:], AF.Exp, scale=1.0 / WS)

            Qb = wkp.tile([128, T + 1], BF16, tag="Qb")
            Pb = wkp.tile([128, T + 1], BF16, tag="Pb")
            nc.gpsimd.memset(Qb[:, 0:1], 0.0)
            nc.vector.memset(Pb[:, 0:1], 0.0)
            nc.vector.tensor_tensor_scan(Qb[:, 1:513], dbt[hh][:],
                                         e[:, 0:512], 0.0, op0=OP.mult,
                                         op1=OP.add)
            nc.vector.tensor_tensor_scan(Qb[:, 513:T + 1], dbt[hh][:],
                                         e[:, 512:T], Qb[:, 512:513],
                                         op0=OP.mult, op1=OP.add)
            Dt = wkp.tile([128, T], BF16, tag="eue")
            nc.vector.scalar_tensor_tensor(Dt[:], e[:], eu_c[hh][:],
                                           Qb[:, 0:T], op0=OP.mult,
                                           op1=OP.add)
            rp = mm_shift(wr8a, wr8b, xn8, hh)
            es1 = wkq.tile([128, T], BF16, tag="es1")
            nc.scalar.activation(es1[:], rp[:], AF.Exp, scale=-1.0 / WS)
            vp = mm_shift(wv8a, wv8b, xn8, hh)
            nc.vector.tensor_tensor(e[:], e[:], vp[:], op=OP.mult)  # ev'
            nc.vector.tensor_tensor_scan(Pb[:, 1:513], dbt[hh][:],
                                         e[:, 0:512], 0.0, op0=OP.mult,
                                         op1=OP.add)
            nc.vector.tensor_tensor_scan(Pb[:, 513:T + 1], dbt[hh][:],
                                         e[:, 512:T], Pb[:, 512:513],
                                         op0=OP.mult, op1=OP.add)
            Np = wkp.tile([128, T], BF16, tag="evu")
            nc.vector.scalar_tensor_tensor(Np[:], e[:], eu_c[hh][:],
                                           Pb[:, 0:T], op0=OP.mult,
                                           op1=OP.add)
            D2 = wkq.tile([128, T], F32, tag="D2")
            nc.vector.scalar_tensor_tensor(D2[:], es1[:], 1.0, Dt[:],
                                           op0=OP.add, op1=OP.mult)
            nc.vector.reciprocal_approx_fast(D2[:], D2[:])
            nc.gpsimd.tensor_tensor(s8[hh // 2][:, hh % 2, :], Np[:], D2[:],
                                    op=OP.mult)

    def p1a(b):
        st[b]["s8"] = [sp_.tile([128, 2, T], FP8, tag=f"s8_{j}",
                                name=f"s8_{j}") for j in range(2)]
        wkv_half(b, (0, 1))

    def p1b(b):
        wkv_half(b, (2, 3))

    def p2(b):   # Wo + residual + LN2
        xt, s8 = st[b]["xt"], st[b]["s8"]
        x1t = []
        for cc in range(CC):
            wop = mm_pair(wo8, s8, cc)
            t_ = x1p.tile([128, T], BF16, tag=f"x1_{cc}")
            nc.vector.scalar_tensor_tensor(t_[:], wop[:], 1.0 / (WS * WS),
                                           xt[cc][:], op0=OP.mult, op1=OP.add)
            x1t.append(t_)
        st[b]["x1t"] = x1t
        st[b]["xn28"] = layer_norm(x1t, "b")

    def kk_half(b, hhs):
        xn28, kk8 = st[b]["xn28"], st[b]["kk8"]
        for hh in hhs:
            ckp = mm_shift(cwk8a, cwk8b, xn28, hh)
            khr = sqp.tile([128, T], BF16, tag="khr")
            nc.scalar.activation(khr[:], ckp[:], AF.Relu)
            if hh % 2 == 0:
                nc.vector.tensor_tensor(kk8[hh // 2][:, hh % 2, :], khr[:],
                                        khr[:], op=OP.mult)
            else:
                nc.scalar.activation(kk8[hh // 2][:, hh % 2, :], khr[:],
                                     AF.Square)

    def p3a(b):
        st[b]["kk8"] = [kkp.tile([128, 2, T], FP8, tag=f"kk8_{j}",
                                 name=f"kk8_{j}") for j in range(HC // 2)]
        kk_half(b, range(0, 8))

    def p3b(b):
        kk_half(b, range(8, HC))

    def p4(b):   # gate + cwv + output
        xn28, kk8, x1t = st[b]["xn28"], st[b]["kk8"], st[b]["x1t"]
        for cc in range(CC):
            r2p = mm_shift(cwr8a, cwr8b, xn28, cc)
            es2 = cmp_.tile([128, T], BF16, tag="es2")
            nc.scalar.activation(es2[:], r2p[:], AF.Exp, scale=-1.0 / WS)
            sig2 = cmp_.tile([128, T], F32, tag="sig2")
            nc.scalar.activation(sig2[:], es2[:], AF.Identity, bias=c16_t[:],
                                 scale=WS * KS * KS)
            nc.vector.reciprocal_approx_fast(sig2[:], sig2[:])
            kvp = mm_pair(cwv8, kk8, cc)
            t1 = cmp_.tile([128, T], BF16, tag="t1")
            nc.vector.scalar_tensor_tensor(t1[:], kvp[:], 1.0, sig2[:],
                                           op0=OP.mult, op1=OP.mult)
            nc.vector.tensor_tensor(t1[:], t1[:], x1t[cc][:], op=OP.add)
            nc.sync.dma_start(y_d[b, cc * 128:(cc + 1) * 128, :], t1[:])

    phases = [p0, p1a, p1b, p2, p3a, p3b, p4]
    NP = len(phases)
    SKEW = 2
    for t in range(NP + SKEW * (bl - 1)):
        for b in range(bl - 1, -1, -1):   # older batches first? no: higher b
            pass
        for b in range(bl):
            p = t - SKEW * b
            if 0 <= p < 5:
                pass
        # emit descending phase index (older batch first) within the step
        work = [(t - SKEW * b, b) for b in range(bl)
                if 0 <= t - SKEW * b < NP]
        for p, b in sorted(work, reverse=True):
            phases[p](b)


def build_program(bl=BL):
    nc = bacc.Bacc("TRN2", target_bir_lowering=False, debug=False,
                   num_devices=NCORES)
    io = {}
    io["x"] = nc.dram_tensor("x", [bl, C, T], BF16, kind="ExternalInput")
    io["y"] = nc.dram_tensor("y", [bl, C, T], BF16, kind="ExternalOutput")
    for nm, npairs, cols in [("wk8a", 2, C), ("wk8b", 2, C), ("wv8a", 2, C),
                             ("wv8b", 2, C), ("wr8a", 2, C), ("wr8b", 2, C),
                             ("wo8", 2, C), ("cwk8a", 2, H), ("cwk8b", 2, H),
                             ("cwv8", 8, C), ("cwr8a", 2, C),
                             ("cwr8b", 2, C)]:
        io[nm] = nc.dram_tensor(nm, [npairs, 128, 2, cols], FP8,
                                kind="ExternalInput")
    for nm in ["delta", "eu"]:
        io[nm] = nc.dram_tensor(nm, [C], F32, kind="ExternalInput")

    with tile.TileContext(nc) as tc:
        with ExitStack() as ctx:
            _emit(nc, tc, ctx, io, bl)
    nc.compile()
    return nc


def _pack_pairs(wT, scale):
    """wT [K, M] contraction-major -> [K//256, 128, 2, M] fp8, slot i of pair
    j = contraction chunk 2j+i."""
    K, M = wT.shape
    out = np.empty((K // 256, 128, 2, M), np.float32)
    for j in range(K // 256):
        for i in range(2):
            out[j, :, i, :] = wT[(2 * j + i) * 128:(2 * j + i + 1) * 128, :]
    out = np.clip(out * scale, -224.0, 224.0)
    return np.ascontiguousarray(out.astype(ml_dtypes.float8_e4m3))


def host_params(inputs):
    """Host-side parameter prep (O(C^2) transposes/folds only)."""
    f32 = np.float32
    g1 = np.asarray(inputs["ln1_g"], f32)
    b1 = np.asarray(inputs["ln1_b"], f32)
    g2 = np.asarray(inputs["ln2_g"], f32)
    b2 = np.asarray(inputs["ln2_b"], f32)
    assert np.allclose(b1, 0.0, atol=1e-30), "nonzero ln1_b not supported"
    assert np.allclose(b2, 0.0, atol=1e-30), "nonzero ln2_b not supported"
    Wk = np.asarray(inputs["Wk"], f32)
    Wv = np.asarray(inputs["Wv"], f32)
    Wr = np.asarray(inputs["Wr"], f32)
    Wo = np.asarray(inputs["Wo"], f32)
    cWk = np.asarray(inputs["cWk"], f32)
    cWr = np.asarray(inputs["cWr"], f32)
    cWv = np.asarray(inputs["cWv"], f32)
    tmk = np.asarray(inputs["tm_k"], f32)[:, None]
    tmv = np.asarray(inputs["tm_v"], f32)[:, None]
    tmr = np.asarray(inputs["tm_r"], f32)[:, None]
    cmk = np.asarray(inputs["cm_k"], f32)[:, None]
    cmr = np.asarray(inputs["cm_r"], f32)[:, None]

    wkT = Wk.T * g1[:, None]
    wvT = Wv.T * g1[:, None]
    wrT = Wr.T * g1[:, None]
    cwkT = cWk.T * g2[:, None]
    cwrT = cWr.T * g2[:, None]

    p = {
        "wk8a": _pack_pairs(wkT * tmk, WS),
        "wk8b": _pack_pairs(wkT * (1.0 - tmk), WS),
        "wv8a": _pack_pairs(wvT * tmv, WS),
        "wv8b": _pack_pairs(wvT * (1.0 - tmv), WS),
        "wr8a": _pack_pairs(wrT * tmr, WS),
        "wr8b": _pack_pairs(wrT * (1.0 - tmr), WS),
        "wo8": _pack_pairs(Wo.T, WS),
        "cwk8a": _pack_pairs(cwkT * cmk, KS),
        "cwk8b": _pack_pairs(cwkT * (1.0 - cmk), KS),
        "cwv8": _pack_pairs(cWv.T, WS),
        "cwr8a": _pack_pairs(cwrT * cmr, WS),
        "cwr8b": _pack_pairs(cwrT * (1.0 - cmr), WS),
        "delta": np.exp(-np.exp(np.asarray(inputs["time_decay"], f32))),
        "eu": np.exp(np.asarray(inputs["time_first"], f32)),
    }
    return p


def host_x(x_sub):
    """[n, T, C] f32 -> [n, C, T] bf16 (layout B)."""
    return np.ascontiguousarray(
        x_sub.transpose(0, 2, 1).astype(ml_dtypes.bfloat16))


def host_y(y_dev):
    """[n, C, T] bf16 -> [n, T, C] f32."""
    return np.asarray(y_dev).astype(np.float32).transpose(0, 2, 1)


def make_in_maps(inputs):
    p = host_params(inputs)
    x = np.asarray(inputs["x"], np.float32)
    return [dict(p, x=host_x(x[c * BL:(c + 1) * BL])) for c in range(NCORES)]


_CACHE = {}


def kernel(**inputs):
    from concourse.bass_utils import run_bass_kernel_spmd

    if "nc" not in _CACHE:
        _CACHE["nc"] = build_program(BL)
    nc = _CACHE["nc"]

    in_maps = make_in_maps(inputs)
    res = run_bass_kernel_spmd(nc, in_maps, list(range(NCORES)))
    out = np.concatenate([host_y(res.results[c]["y"]) for c in range(NCORES)],
                         axis=0)
    return np.ascontiguousarray(out.astype(np.float32))
